# revision 16
# baseline (speedup 1.0000x reference)
"""Trainium2 Bass kernel for nn_AttentionBlock (LN -> QKV -> softmax attn -> proj + residual).

Sharding: x rows (sequence) across 8 cores. Each core receives the FULL x,
rolled so that its own 1024 query rows come first; it recomputes K (and the
z = layernormed x rows used in place of V) for all 8192 keys locally — this
replicated compute is cheaper than all-gathering K/V through HBM+collectives.

Math folding (host side, exact or fp32-benign):
  xn = (x - mu) * rsig * norm_w + norm_b
  qkv = xn @ qkv_w.T + qkv_b
     => with W' = qkv_w * norm_w, b' = qkv_w @ norm_b + qkv_b and
        z = (x - mu) * rsig:      q/k/v = z @ W'.T + b'
  V-bias bv and the attention-value projection fold through the output proj:
     attn @ v @ proj_w.T = (attn @ z) @ (proj_w @ wv').T   (+ const terms)
  so the kernel never materializes V: it computes R = softmax(S) @ z and then
  F = R @ (proj_w @ wv')^T + (proj_b + bv @ proj_w.T) + x.

On-chip dataflow per core (all matmuls fp32r, free-dim >= 256):
  phase 1/2 (streamed in 16 groups of 512 rows):
    x chunk -> bn_stats/bn_aggr -> rsig = exp(-0.5*ln(var+eps)) (keeps a single
    ACT table: {ln, exp, identity}) -> z rows (row-major, with a 257th column
    of ones) -> PE-transpose z -> znT chunk -> K^T chunk (and Q^T for rows
    0:1024) via matmul with pre-transposed weights.
  phase 3 (2 query chunks of 512):
    for each of 64 key tiles: S^T = K^T_tile.T @ Q^T (PSUM), P^T = exp(S^T/16)
    (ACT, PSUM->SBUF), R += P^T.T @ [z | 1] accumulated in PSUM. Column 256 of
    R is the softmax denominator. Normalize, PE-transpose R, one matmul with
    the folded projection, add residual + folded bias, DMA out.
"""

import os

import numpy as np

N = 8192
D = 256
NCORES = 8
M = N // NCORES          # 1024 query rows per core
NT = N // 128            # 64 key tiles
GROUP = 4                # key tiles per phase-1 group (512 rows)
NGROUPS = NT // GROUP    # 16
MCHUNK = 512             # query columns per phase-3 chunk
NMC = M // MCHUNK        # 2
EPS = 1e-5
SCALE = 1.0 / 16.0       # 1/sqrt(D)
FP8_S = os.environ.get("KERNEL_FP8_S", "1") == "1"  # fp8e4m3+DoubleRow scores


def build():
    import concourse.bacc as bacc
    import concourse.bass as bass
    import concourse.tile as tile
    from concourse import mybir
    from concourse.masks import make_identity

    f32 = mybir.dt.float32
    f32r = mybir.dt.float32r
    AF = mybir.ActivationFunctionType
    ALU = mybir.AluOpType

    # Force the ACT table-load pass to use the one table that contains all
    # of {Identity, Copy, Exp, Ln} (natural_log_exp_and_others). The stock
    # pass greedily picks the first matching set per function, alternating
    # exp_and_others/natural_log and paying a 1.3us table reload each flip.
    from concourse.hw_specs import get_activation_tables as _gat
    def _one_table(arch):
        return {name: (funcs if name == "natural_log_exp_and_others" else set())
                for name, funcs in _gat(arch).items()}
    bacc.get_activation_tables = _one_table

    nc = bacc.Bacc("TRN2", target_bir_lowering=False, debug=False,
                   num_devices=NCORES)

    x = nc.dram_tensor("x", [N, D], f32, kind="ExternalInput").ap()
    wqk = nc.dram_tensor("wqk", [D, 2 * D], f32r, kind="ExternalInput").ap()
    wfold = nc.dram_tensor("wfold", [D, D], f32r, kind="ExternalInput").ap()
    bqk = nc.dram_tensor("bqk", [2, D], f32, kind="ExternalInput").ap()
    pb2 = nc.dram_tensor("pb2", [D], f32, kind="ExternalInput").ap()
    out = nc.dram_tensor("out", [M, D], f32, kind="ExternalOutput").ap()

    x_r = x.rearrange("(t p) d -> p t d", p=128)        # [128, 64, 256]
    out_r = out.rearrange("(t p) d -> p t d", p=128)    # [128, 8, 256]

    with tile.TileContext(nc) as tc, \
            tc.tile_pool(name="singles", bufs=1) as singles:
        # ---- persistent tensors ----
        z_t = singles.tile([128, NT, D + 2], f32r, name="z_t")       # z rows + ones
        kq_dt = mybir.dt.float8e4 if FP8_S else f32r
        kT = singles.tile([128, 2, N], kq_dt, name="kT")             # K^T, 2 d-halves
        qT = singles.tile([128, 2, M], kq_dt, name="qT")             # Q^T, 2 d-halves
        wqk_sb = singles.tile([128, 2, 2 * D], f32r, name="wqk_sb")  # [c_half, (q|k) d]
        wfold_sb = singles.tile([128, 2, D], f32r, name="wfold_sb")  # [c_half, e]
        bqk_sb = singles.tile([128, 2, 2], f32, name="bqk_sb")      # [d, (q|k), d_half]
        pb_sb = singles.tile([128, D], f32, name="pb_sb")
        xop = singles.tile([128, M // 128, D], f32, name="xop")     # residual + pb2
        ident_f = singles.tile([128, 128], f32, name="ident_f")
        ident = singles.tile([128, 128], f32r, name="ident")
        eps_t = singles.tile([128, 1], f32, name="eps_t")

        nc.sync.dma_start(out=wqk_sb, in_=wqk.rearrange("(ch p) d -> p ch d", p=128))
        nc.sync.dma_start(out=wfold_sb, in_=wfold.rearrange("(ch p) e -> p ch e", p=128))
        nc.sync.dma_start(out=bqk_sb, in_=bqk.rearrange("k (h p) -> p k h", p=128))
        pb2_bc = bass.AP(tensor=pb2.tensor, offset=pb2.offset,
                         ap=[[0, 128]] + list(pb2.ap))
        nc.sync.dma_start(out=pb_sb, in_=pb2_bc)
        nc.sync.dma_start(out=xop, in_=x_r[:, 0:M // 128, :])
        ones_f = singles.tile([128, 2 * NT], f32, name="ones_f")
        make_identity(nc, ident_f)
        nc.scalar.copy(ident, ident_f)
        nc.vector.memset(eps_t, EPS)
        nc.vector.memset(ones_f, 1.0)
        nc.scalar.activation(
            out=z_t[:, :, D:D + 2],
            in_=ones_f.rearrange("p (t o) -> p t o", o=2), func=AF.Identity)
        for t in range(M // 128):
            nc.vector.tensor_add(xop[:, t, :], xop[:, t, :], pb_sb)

        with (
            tc.tile_pool(name="xg_pool", bufs=3) as xg_pool,
            tc.tile_pool(name="st_pool", bufs=2) as st_pool,
            tc.tile_pool(name="znT_pool", bufs=2) as znT_pool,
            tc.tile_pool(name="pT_pool", bufs=3) as pT_pool,
            tc.tile_pool(name="rn_pool", bufs=4) as rn_pool,
            tc.tile_pool(name="rnT_pool", bufs=2) as rnT_pool,
            tc.tile_pool(name="f_pool", bufs=3) as f_pool,
            tc.tile_pool(name="tp_psum", bufs=2, space="PSUM") as tp_psum,
            tc.tile_pool(name="mm_psum", bufs=2, space="PSUM") as mm_psum,
            tc.tile_pool(name="r_psum", bufs=4, space="PSUM") as r_psum,
        ):
            # ---------------- phase 1+2: LN, transpose, K^T / Q^T ----------
            for g in range(NGROUPS if os.environ.get("KBUILD_PH12", "1") == "1" else 0):
                xg = xg_pool.tile([128, GROUP, D], f32, name=f"xg{g}", tag="xg")
                nc.sync.dma_start(out=xg, in_=x_r[:, g * GROUP:(g + 1) * GROUP, :])

                st = st_pool.tile([128, GROUP, 6], f32, name=f"st{g}", tag="st")
                mv = st_pool.tile([128, GROUP, 2], f32, name=f"mv{g}", tag="mv")
                for j in range(GROUP):
                    nc.vector.bn_stats(out=st[:, j, :], in_=xg[:, j, :])
                for j in range(GROUP):
                    nc.vector.bn_aggr(out=mv[:, j, :], in_=st[:, j, :])
                # rsig = exp(-0.5 * ln(var + eps)); table stays {ln,exp,identity}
                lnv = st_pool.tile([128, GROUP], f32, name=f"lnv{g}", tag="lnv")
                rsig = st_pool.tile([128, GROUP], f32, name=f"rsig{g}", tag="rsig")
                nmr = st_pool.tile([128, GROUP], f32, name=f"nmr{g}", tag="nmr")
                nc.scalar.activation(
                    out=lnv, in_=mv[:, :, 1:2].rearrange("p t o -> p (t o)"),
                    func=AF.Ln, bias=eps_t, scale=1.0)
                nc.scalar.activation(out=rsig, in_=lnv, func=AF.Exp, scale=-0.5)
                nc.vector.tensor_scalar_mul(
                    nmr, mv[:, :, 0:1].rearrange("p t o -> p (t o)"), -1.0)

                for j in range(GROUP):
                    t = g * GROUP + j
                    nc.gpsimd.tensor_scalar(
                        out=z_t[:, t, 0:D], in0=xg[:, j, :],
                        scalar1=nmr[:, j:j + 1], scalar2=rsig[:, j:j + 1],
                        op0=ALU.add, op1=ALU.mult)

                znT = znT_pool.tile([128, 2, GROUP * 128], f32r,
                                    name=f"znT{g}", tag="znT")
                for ch in range(2):
                    tp = tp_psum.tile([128, GROUP * 128], f32r,
                                      name=f"tpz{g}_{ch}", tag="tp")
                    for j in range(GROUP):
                        nc.tensor.transpose(
                            tp[:, j * 128:(j + 1) * 128],
                            z_t[:, g * GROUP + j, ch * 128:(ch + 1) * 128],
                            ident)
                    if ch == 0:
                        nc.vector.tensor_copy(znT[:, ch, :], tp)
                    else:
                        nc.scalar.activation(out=znT[:, ch, :], in_=tp,
                                             func=AF.Identity)

                for dh in range(2):
                    kp = mm_psum.tile([128, GROUP * 128], f32,
                                      name=f"kp{g}_{dh}", tag="mm")
                    for ch in range(2):
                        nc.tensor.matmul(
                            kp,
                            lhsT=wqk_sb[:, ch, D + dh * 128:D + (dh + 1) * 128],
                            rhs=znT[:, ch, :],
                            start=(ch == 0), stop=(ch == 1))
                    nc.scalar.activation(
                        out=kT[:, dh, g * GROUP * 128:(g + 1) * GROUP * 128],
                        in_=kp, func=AF.Identity, bias=bqk_sb[:, 1, dh:dh + 1])

                if g * GROUP * 128 < M:
                    for dh in range(2):
                        qp = mm_psum.tile([128, GROUP * 128], f32,
                                          name=f"qp{g}_{dh}", tag="mm")
                        for ch in range(2):
                            nc.tensor.matmul(
                                qp,
                                lhsT=wqk_sb[:, ch, dh * 128:(dh + 1) * 128],
                                rhs=znT[:, ch, :],
                                start=(ch == 0), stop=(ch == 1))
                        nc.vector.tensor_scalar(
                            out=qT[:, dh, g * GROUP * 128:(g + 1) * GROUP * 128],
                            in0=qp, scalar1=bqk_sb[:, 0, dh:dh + 1], scalar2=None,
                            op0=ALU.add)

            # ---------------- phase 3: attention ---------------------------
            for mc in range(NMC if os.environ.get("KBUILD_PH3", "1") == "1" else 0):
                rps = [r_psum.tile([128, D + 2], f32, name=f"rps{mc}_{mt}",
                                   tag="rps") for mt in range(MCHUNK // 128)]

                def emit_s_exp(t, mc=mc):
                    sp = mm_psum.tile([128, MCHUNK], f32,
                                      name=f"sp{mc}_{t}", tag="mm")
                    if FP8_S:
                        nc.tensor.matmul(
                            sp,
                            lhsT=kT[:, :, t * 128:(t + 1) * 128],
                            rhs=qT[:, :, mc * MCHUNK:(mc + 1) * MCHUNK],
                            start=True, stop=True,
                            perf_mode=mybir.MatmulPerfMode.DoubleRow)
                    else:
                        for dh in range(2):
                            nc.tensor.matmul(
                                sp,
                                lhsT=kT[:, dh, t * 128:(t + 1) * 128],
                                rhs=qT[:, dh, mc * MCHUNK:(mc + 1) * MCHUNK],
                                start=(dh == 0), stop=(dh == 1))
                    pT = pT_pool.tile([128, MCHUNK], f32r,
                                      name=f"pT{mc}_{t}", tag="pT")
                    nc.scalar.activation(out=pT, in_=sp, func=AF.Exp, scale=SCALE)
                    return pT

                pT_cur = emit_s_exp(0)
                for t in range(NT):
                    pT_next = emit_s_exp(t + 1) if t + 1 < NT else None
                    for mt in range(MCHUNK // 128):
                        nc.tensor.matmul(
                            rps[mt],
                            lhsT=pT_cur[:, mt * 128:(mt + 1) * 128],
                            rhs=z_t[:, t, :],
                            start=(t == 0), stop=(t == NT - 1))
                    pT_cur = pT_next

                rn_tiles = []
                for mt in range(MCHUNK // 128):
                    den = st_pool.tile([128, 1], f32, name=f"den{mc}_{mt}",
                                       tag="den", bufs=4)
                    nc.vector.reciprocal(den, rps[mt][:, D:D + 1])
                    rn = rn_pool.tile([128, D], f32r, name=f"rn{mc}_{mt}",
                                      tag="rn")
                    nc.vector.tensor_scalar_mul(rn, rps[mt][:, 0:D], den)
                    rn_tiles.append(rn)
                rnT = rnT_pool.tile([128, 2, MCHUNK], f32r, name=f"rnT{mc}",
                                    tag="rnT")
                for ch in range(2):
                    tp = tp_psum.tile([128, MCHUNK], f32r,
                                      name=f"tpr{mc}_{ch}", tag="tp")
                    for mt in range(MCHUNK // 128):
                        nc.tensor.transpose(
                            tp[:, mt * 128:(mt + 1) * 128],
                            rn_tiles[mt][:, ch * 128:(ch + 1) * 128], ident)
                    nc.vector.tensor_copy(rnT[:, ch, :], tp)
                for mt in range(MCHUNK // 128):
                    fp = mm_psum.tile([128, D], f32, name=f"fp{mc}_{mt}",
                                      tag="mm")
                    for ch in range(2):
                        nc.tensor.matmul(
                            fp,
                            lhsT=rnT[:, ch, mt * 128:(mt + 1) * 128],
                            rhs=wfold_sb[:, ch, :],
                            start=(ch == 0), stop=(ch == 1))
                    mtg = mc * (MCHUNK // 128) + mt
                    fs = f_pool.tile([128, D], f32, name=f"fs{mc}_{mt}",
                                     tag="fs")
                    nc.vector.tensor_add(fs, fp, xop[:, mtg, :])
                    nc.sync.dma_start(out=out_r[:, mtg, :], in_=fs)

    nc.compile()
    return nc


def _host_fold(norm_w, norm_b, qkv_w, qkv_b, proj_w, proj_b):
    W = (qkv_w * norm_w[None, :]).astype(np.float32)
    bqkv = (qkv_w @ norm_b + qkv_b).astype(np.float32)
    wq, wk, wv = W[0:D], W[D:2 * D], W[2 * D:3 * D]
    bq, bk, bv = bqkv[0:D], bqkv[D:2 * D], bqkv[2 * D:3 * D]
    wqk = np.ascontiguousarray(np.concatenate([wq.T, wk.T], axis=1))  # [D, 2D]
    wfold = np.ascontiguousarray((proj_w @ wv).T)                     # [D, D]
    bqk = np.ascontiguousarray(np.stack([bq, bk], axis=0))            # [2, D]
    pb2 = np.ascontiguousarray(proj_b + bv @ proj_w.T)                # [D]
    return wqk, wfold, bqk, pb2


def make_in_maps(x, norm_w, norm_b, qkv_w, qkv_b, proj_w, proj_b):
    x = np.ascontiguousarray(np.asarray(x, dtype=np.float32))
    wqk, wfold, bqk, pb2 = _host_fold(
        np.asarray(norm_w, np.float32), np.asarray(norm_b, np.float32),
        np.asarray(qkv_w, np.float32), np.asarray(qkv_b, np.float32),
        np.asarray(proj_w, np.float32), np.asarray(proj_b, np.float32))
    in_maps = []
    for i in range(NCORES):
        xi = np.ascontiguousarray(np.roll(x, -i * M, axis=0))
        in_maps.append({"x": xi, "wqk": wqk, "wfold": wfold, "bqk": bqk,
                        "pb2": pb2})
    return in_maps


def kernel(x, norm_w, norm_b, qkv_w, qkv_b, proj_w, proj_b):
    from concourse.bass_utils import run_bass_kernel_spmd

    nc = build()
    in_maps = make_in_maps(x, norm_w, norm_b, qkv_w, qkv_b, proj_w, proj_b)
    trace = bool(int(os.environ.get("KERNEL_TRACE", "0")))
    res = run_bass_kernel_spmd(nc, in_maps, core_ids=list(range(NCORES)),
                               trace=trace)
    if trace and res.exec_time_ns is not None:
        print(f"KERNEL exec_time_ns={res.exec_time_ns} "
              f"mean={res.mean_exec_time_ns} trace={res.instructions_and_trace[1] if res.instructions_and_trace else None}")
    return np.ascontiguousarray(
        np.concatenate([res.results[i]["out"] for i in range(NCORES)], axis=0))
